# revision 40
# baseline (speedup 1.0000x reference)
"""Multi-head attention (B=8, S=1024, D=1024, H=16) on 8 TRN2 NeuronCores.

Sharding: pure data parallel — batch element b on core b. Weights are
broadcast to every core. No collectives.

v3: engine-balanced + fp8 DoubleRow on the V side.

  - ACT does ONLY exp (128 x [128,1024] = its ~141us floor). All
    PSUM->SBUF drains / bias-adds / casts run on DVE.
  - Normalization uses reciprocal_approx_fast (~5x cheaper than
    nc.vector.reciprocal, which cost 2.6us per [1,512] row in v1).
  - V projection and PV run in fp8e4 with perf_mode=DoubleRow: each
    matmul contracts 256 (2 k-planes via 3D [128,2,N] APs), halving
    instruction count. V/ex quantization noise washes out through the
    softmax average (measured: adds ~1e-3 to rel err). Q/K projection,
    scores, and the output projection stay bf16.
  - Emission order respects dataflow (Tile derives dependencies from
    trace order!) while ramping ACT early: X tiles 0-3 -> V(0..2) ->
    QK(pair0, first half) -> attention(pair0, sk 0-3) -> rest of X/V ->
    attention(pair0, rest) -> pairs 1-7. The priority heap gap-fills PE
    with later-emitted B/C/E matmuls while D waits on exp.

Per-core algorithm (X: [S, D] for one batch element):
  1. X^T via PE transposes (bf16 matmul vs identity); kept as bf16
     (for QK proj rhs) and fp8e4 (for V proj DoubleRow lhsT).
  2. QK^T = W_in[:, :2D]^T @ X -> [2D, S] bf16, head-dim-on-partitions.
  3. V = X @ W_in[:, 2D:] (fp8 DoubleRow), stored fp8 as
     V_aug[skpair, j, head, 65] with a ones column (col 64) so PV also
     produces the softmax denominator row.
  4. Per head pair: S^T = K_h^T.T @ Q_h^T (bf16, PE row-groups 0-63 /
     64-127 concurrently for the 2 heads), wide exp on ACT -> fp8
     (scale=1/8, max-subtraction skipped: scores ~N(0,1), exp <= e^6),
     PV fp8 DoubleRow over sk-pairs accumulates out^T + denominator.
  5. Normalize: reciprocal_approx_fast on row 64, GpSimd broadcast,
     DVE multiply -> attnT (bf16).
  6. Y = attn_out @ W_out + b_out in bf16.
"""

import sys

sys.path.insert(0, "/opt/trn_rl_repo")

import numpy as np

import concourse.bacc as bacc
import concourse.mybir as mybir
from concourse.bass_utils import run_bass_kernel_spmd
from concourse.masks import make_identity
from concourse.tile import TileContext

B = 8
S = 1024
D = 1024
H = 16
DK = D // H  # 64
P = 128
ST = S // P   # 8 s-tiles
DT = D // P   # 8 d-tiles
NTQK = 2 * D // P  # 16 n-tiles for the Q|K part
PAIRS = H // 2     # 8 head pairs
SC = S // 512      # 2 query chunks of 512 (matmul free-dim limit)

f32 = mybir.dt.float32
bf16 = mybir.dt.bfloat16
EXP = mybir.ActivationFunctionType.Exp
MULT = mybir.AluOpType.mult
ADD = mybir.AluOpType.add


def build_nc():
    nc = bacc.Bacc()
    X = nc.dram_tensor("X", [S, D], f32, kind="ExternalInput")
    W_in = nc.dram_tensor("W_in", [D, 3 * D], f32, kind="ExternalInput")
    b_in = nc.dram_tensor("b_in", [3 * D], f32, kind="ExternalInput")
    W_out = nc.dram_tensor("W_out", [D, D], f32, kind="ExternalInput")
    b_out = nc.dram_tensor("b_out", [D], f32, kind="ExternalInput")
    out = nc.dram_tensor("out", [S, D], f32, kind="ExternalOutput")

    w_in_kp = W_in.rearrange("(ko p) n -> p ko n", p=P)  # [128, 8, 3072]
    w_out_kp = W_out.rearrange("(ko p) n -> p ko n", p=P)  # [128, 8, 1024]

    with TileContext(nc) as tc:
        const = tc.alloc_tile_pool(name="const", bufs=1)
        # PSUM: sps 2x[128,1024] (4 banks) + pv 2x[128,512] (2 banks)
        #       + gp 2x[128,512] (2 banks) = 8 banks
        sps_pool = tc.alloc_tile_pool(name="spsp", bufs=2, space="PSUM")
        pv_pool = tc.alloc_tile_pool(name="pvp", bufs=1, space="PSUM")
        gp_pool = tc.alloc_tile_pool(name="gpp", bufs=2, space="PSUM")

        identity = const.tile([P, P], bf16)
        make_identity(nc, identity[:])
        bqk = const.tile([P, NTQK], f32)
        nc.sync.dma_start(bqk[:], b_in[0 : 2 * D].rearrange("(o p) -> p o", p=P))
        bv_bc = const.tile([P, D], f32)
        bout_bc = const.tile([P, D], f32)
        ones_src = const.tile([P, ST * H], f32)
        nc.vector.memset(ones_src[:], 1.0)

        brow_pool = tc.alloc_tile_pool(name="brow", bufs=1)
        bv_row = brow_pool.tile([1, D], f32, tag="row")
        nc.sync.dma_start(bv_row[:], b_in[None, 2 * D : 3 * D])
        nc.gpsimd.partition_broadcast(bv_bc[:], bv_row[:])
        bout_row = brow_pool.tile([1, D], f32, tag="row")
        nc.sync.dma_start(bout_row[:], b_out[None, :])
        nc.gpsimd.partition_broadcast(bout_bc[:], bout_row[:])

        # ---------------- resident tensors ----------------
        xT_pool = tc.alloc_tile_pool(name="xT", bufs=1)
        xT = xT_pool.tile([P, DT, S], bf16)    # 2 MB
        qkT_pool = tc.alloc_tile_pool(name="qkT", bufs=1)
        qkT = qkT_pool.tile([P, NTQK, S], bf16)  # 4 MB
        vaug_pool = tc.alloc_tile_pool(name="vaug", bufs=1)
        v_aug = vaug_pool.tile([P, ST, H, DK + 1], bf16)  # 2.1 MB
        nc.vector.tensor_copy(
            v_aug[:, :, :, DK : DK + 1],
            ones_src[:].rearrange("p (s h one) -> p s h one", h=H, one=1),
        )
        attnT_pool = tc.alloc_tile_pool(name="attnT", bufs=1)
        attnT = attnT_pool.tile([P, DT, S], bf16)  # 2 MB
        wv_pool = tc.alloc_tile_pool(name="wv", bufs=1)
        wv = wv_pool.tile([P, DT, D], bf16)   # 2 MB
        wout_pool = tc.alloc_tile_pool(name="wout", bufs=1)
        wout = wout_pool.tile([P, DT, D], bf16)  # 2 MB

        # staging pools; big weights staged in 2-MB n-halves (SBUF budget)
        wvs_pool = tc.alloc_tile_pool(name="wvs", bufs=2)

        def stage_weight(dst, src_cols, ncx):
            wst = wvs_pool.tile([P, DT, 512], f32, tag="wstage")
            nc.sync.dma_start(wst[:], src_cols)
            for dg in range(2):
                nc.vector.tensor_copy(
                    dst[:, dg * 4 : (dg + 1) * 4, ncx * 512 : (ncx + 1) * 512],
                    wst[:, dg * 4 : (dg + 1) * 4, :],
                )

        xs_pool = tc.alloc_tile_pool(name="xs", bufs=2)
        xb_pool = tc.alloc_tile_pool(name="xb", bufs=2)
        wqs_pool = tc.alloc_tile_pool(name="wqs", bufs=2)
        wqb_pool = tc.alloc_tile_pool(name="wqb", bufs=2)
        ex_pool = tc.alloc_tile_pool(name="exp", bufs=3)
        nrm_pool = tc.alloc_tile_pool(name="nrm", bufs=1)
        y_pool = tc.alloc_tile_pool(name="yp", bufs=2)

        # ---------------- emission helpers ----------------
        def phase_a(st):
            """DMA + cast + PE-transpose X s-tile st into xT."""
            x_tile = xs_pool.tile([P, D], f32, tag="x")
            nc.sync.dma_start(x_tile[:], X[st * P : (st + 1) * P, :])
            xb = xb_pool.tile([P, D], bf16, tag="xb")
            nc.vector.tensor_copy(xb[:], x_tile[:])
            for half in range(2):
                gp = gp_pool.tile([P, 512], f32, tag="gp", name="tpa")
                for j in range(4):
                    dj = half * 4 + j
                    nc.tensor.matmul(
                        gp[:, j * P : (j + 1) * P],
                        xb[:, dj * P : (dj + 1) * P],
                        identity[:],
                        start=True,
                        stop=True,
                    )
                sl = (slice(None), slice(half * 4, (half + 1) * 4),
                      slice(st * P, (st + 1) * P))
                nc.vector.tensor_copy(
                    xT[sl], gp[:].rearrange("p (j s) -> p j s", j=4)
                )

        def phase_c(st):
            """V projection -> v_aug[:, st, :, 0:64] (+bias)."""
            for ncx in range(SC):
                gp = gp_pool.tile([P, 512], f32, tag="gp", name="psc")
                for dk in range(DT):
                    nc.tensor.matmul(
                        gp[:],
                        xT[:, dk, st * P : (st + 1) * P],
                        wv[:, dk, ncx * 512 : (ncx + 1) * 512],
                        start=(dk == 0),
                        stop=(dk == DT - 1),
                    )
                hbase = ncx * (H // SC)
                nc.vector.tensor_tensor(
                    v_aug[:, st, hbase : hbase + H // SC, 0:DK],
                    gp[:].rearrange("p (h d) -> p h d", d=DK),
                    bv_bc[:, ncx * 512 : (ncx + 1) * 512].rearrange(
                        "p (h d) -> p h d", d=DK
                    ),
                    ADD,
                )

        def phase_b_half(nt, sc, w_tile):
            """QK^T projection for n-tile nt, query chunk sc."""
            gp = gp_pool.tile([P, 512], f32, tag="gp", name="psb")
            for dk in range(DT):
                nc.tensor.matmul(
                    gp[:],
                    w_tile[:, dk, :],
                    xT[:, dk, sc * 512 : (sc + 1) * 512],
                    start=(dk == 0),
                    stop=(dk == DT - 1),
                )
            nc.vector.tensor_scalar(
                qkT[:, nt, sc * 512 : (sc + 1) * 512],
                gp[:],
                bqk[:, nt : nt + 1],
                None,
                ADD,
            )

        def phase_b_load(nt):
            w_stage = wqs_pool.tile([P, DT, P], f32, tag="ws")
            nc.sync.dma_start(w_stage[:], w_in_kp[:, :, nt * P : (nt + 1) * P])
            w_tile = wqb_pool.tile([P, DT, P], bf16, tag="w")
            nc.vector.tensor_copy(w_tile[:], w_stage[:])
            return w_tile

        def phase_b(nt):
            w_tile = phase_b_load(nt)
            for sc in range(SC):
                phase_b_half(nt, sc, w_tile)

        def d_scores(pr, sc, sk):
            """Paired score matmuls + exp for key tile sk -> fp8 ex plane."""
            sps = sps_pool.tile([P, S], f32, tag="sps", name="sps")
            for hh in range(2):
                base = hh * DK
                nc.tensor.matmul(
                    sps[:, hh * 512 : (hh + 1) * 512],
                    qkT[base : base + DK, PAIRS + pr, sk * P : (sk + 1) * P],
                    qkT[base : base + DK, pr, sc * 512 : (sc + 1) * 512],
                    start=True,
                    stop=True,
                )
            return sps

        def phase_d(pr, sc, sk_range, state):
            """Attention pipeline chunk for head pair pr, query chunk sc."""
            if state is None:
                state = {
                    "pv": [
                        pv_pool.tile([P, 512], f32, tag=f"pv{i}", name=f"pv{i}")
                        for i in range(2)
                    ],
                    "ex": {},
                }
            pv, exs = state["pv"], state["ex"]
            for sk in sk_range:
                if sk < ST:
                    sps = d_scores(pr, sc, sk)
                    ex = ex_pool.tile([P, S], bf16, tag="ex", name="ex")
                    nc.scalar.activation(
                        ex[:], sps[:], EXP, scale=1.0 / np.sqrt(DK)
                    )
                    exs[sk] = ex
                if sk >= 1:
                    ex = exs.pop(sk - 1)
                    for hh in range(2):
                        h = 2 * pr + hh
                        nc.tensor.matmul(
                            pv[hh][0 : DK + 1, :],
                            v_aug[:, sk - 1, h, :],
                            ex[:, hh * 512 : (hh + 1) * 512],
                            start=(sk - 1 == 0),
                            stop=(sk - 1 == ST - 1),
                        )
            return state

        def d_normalize(pr, sc, state):
            for hh in range(2):
                base = hh * DK
                pvh = state["pv"][hh]
                # one copy frees the PSUM slot for the next pair's PV
                # immediately; the normalize chain reads the SBUF copy
                # (reciprocal_approx_fast needs SBUF input anyway — the
                # custom-DVE op reads garbage from PSUM)
                pvc = nrm_pool.tile([DK + 1, 512], f32, tag="pvc", name="pvc")
                nc.vector.tensor_copy(pvc[:], pvh[0 : DK + 1, :])
                den = nrm_pool.tile([1, 512], f32, tag="dr", name="den")
                nc.vector.tensor_copy(den[:], pvc[DK : DK + 1, :])
                rrow = nrm_pool.tile([1, 512], f32, tag="rr", name="rrow")
                nc.vector.reciprocal_approx_fast(rrow[:], den[:])
                # full-tile broadcast (sliced outputs break on HW)
                bc = nrm_pool.tile([P, 512], f32, tag="bc", name="bc")
                nc.gpsimd.partition_broadcast(bc[:], rrow[:])
                nc.vector.tensor_tensor(
                    attnT[base : base + DK, pr, sc * 512 : (sc + 1) * 512],
                    pvc[0:DK, :],
                    bc[0:DK, :],
                    MULT,
                )

        def phase_e(st):
            """Output projection for s-tile st."""
            for ncx in range(SC):
                gp = gp_pool.tile([P, 512], f32, tag="gp", name="pse")
                for dk in range(DT):
                    nc.tensor.matmul(
                        gp[:],
                        attnT[:, dk, st * P : (st + 1) * P],
                        wout[:, dk, ncx * 512 : (ncx + 1) * 512],
                        start=(dk == 0),
                        stop=(dk == DT - 1),
                    )
                y = y_pool.tile([P, 512], f32, tag="y")
                nc.vector.tensor_tensor(
                    y[:], gp[:], bout_bc[:, ncx * 512 : (ncx + 1) * 512], ADD
                )
                nc.sync.dma_start(
                    out[st * P : (st + 1) * P, ncx * 512 : (ncx + 1) * 512],
                    y[:],
                )

        # ------------- emission order (must respect dataflow!) -------------
        # The Tile scheduler freezes ONE linear order per engine from a
        # cost-model simulation — no runtime gap-filling, and fine-grained
        # interleaving of unrelated matmul streams inflates LDWEIGHTS/sync
        # cost by ~25% (measured). So work stays in contiguous blocks, and
        # the blocks are ordered so no engine FIFO entry waits on late data.
        #
        # HAM: the PE clock starts throttled (1.2 GHz) and needs ~3.4us of
        # sustained activity to reach 2.4 GHz; ~5us of identity matmuls at
        # t=0 burn the X-DMA dead time to warm it up.
        for wu in range(26):
            gpw = gp_pool.tile([P, 512], f32, tag="gp", name="warm")
            for j in range(4):
                nc.tensor.matmul(
                    gpw[:, j * P : (j + 1) * P],
                    identity[:],
                    identity[:],
                    start=True,
                    stop=True,
                )

        # ramp: X tiles 0-3, pair-0 Q/K weights, then B/scores for the
        # first query half BEFORE the remaining X transposes, so the PE
        # FIFO never stalls behind a DMA-gated entry.
        def keep_warm(n=4):
            gpw = gp_pool.tile([P, 512], f32, tag="gp", name="warm")
            for j in range(n):
                nc.tensor.matmul(
                    gpw[:, j * P : (j + 1) * P],
                    identity[:], identity[:], start=True, stop=True,
                )

        for st in range(4):
            phase_a(st)
            keep_warm()
        w0 = phase_b_load(0)
        w8 = phase_b_load(PAIRS)
        phase_b_half(0, 0, w0)
        phase_b_half(PAIRS, 0, w8)
        st0 = d_new_state()
        d_scores_exp(0, 0, range(0, 4), st0)
        for st in range(4, ST):
            phase_a(st)
            keep_warm()
        phase_b_half(0, 1, w0)
        phase_b_half(PAIRS, 1, w8)
        d_scores_exp(0, 0, range(4, ST), st0)
        # pair-1's QK projection fills the PE while the W_v DMA lands
        # (also keeps HAM from re-throttling mid-ramp)
        w1 = phase_b_load(1)
        w9 = phase_b_load(PAIRS + 1)
        phase_b_half(1, 0, w1)
        phase_b_half(PAIRS + 1, 0, w9)
        phase_b_half(1, 1, w1)
        phase_b_half(PAIRS + 1, 1, w9)
        # W_v first half + the v_aug tiles pair 0 PV needs
        stage_weight(wv, w_in_kp[:, :, 2 * D : 2 * D + 512], 0)
        for st in range(4):
            phase_c(st, ncxs=(0,))
        d_pv(0, 0, range(0, 3), st0)
        for st in range(4, ST):
            phase_c(st, ncxs=(0,))
        d_pv(0, 0, range(3, ST), st0)
        d_normalize(0, 0, st0)
        # second V half (heads 8-15, pairs 4-7)
        stage_weight(wv, w_in_kp[:, :, 2 * D + 512 : 3 * D], 1)
        for st in range(ST):
            phase_c(st, ncxs=(1,))

        for pr in range(1, PAIRS):
            if pr > 1:
                phase_b(pr)
                phase_b(PAIRS + pr)
            state = d_new_state()
            for sk in range(ST + 1):
                if sk < ST:
                    d_scores_exp(pr, 0, [sk], state)
                if sk >= 1:
                    d_pv(pr, 0, [sk - 1], state)
            d_normalize(pr, 0, state)

        # W_out prefetch + cast (DMA during the sc=0 sweep)
        for ncx in range(SC):
            stage_weight(wout, w_out_kp[:, :, ncx * 512 : (ncx + 1) * 512], ncx)

        # sc=1 sweep; E(0..3) interleaves (their attnT halves are complete)
        for pr in range(PAIRS):
            state = d_new_state()
            for sk in range(ST + 1):
                if sk < ST:
                    d_scores_exp(pr, 1, [sk], state)
                if sk >= 1:
                    d_pv(pr, 1, [sk - 1], state)
            d_normalize(pr, 1, state)
            if 1 <= pr <= 4:
                for f in e_closures(pr - 1):
                    f()
        for st in range(4, ST):
            for f in e_closures(st):
                f()

        for pool in (
            y_pool, nrm_pool, ex_pool, wqb_pool, wqs_pool, xb_pool, xs_pool,
            wvs_pool, wout_pool, wv_pool, attnT_pool, vaug_pool, qkT_pool,
            xT_pool, gp_pool, pv_pool, sps_pool, brow_pool, const,
        ):
            pool.release()

    nc.finalize()
    return nc


_NC_CACHE = {}


def get_nc():
    if "nc" not in _NC_CACHE:
        _NC_CACHE["nc"] = build_nc()
    return _NC_CACHE["nc"]


def kernel(X, W_in, b_in, W_out, b_out):
    X = np.ascontiguousarray(np.asarray(X, dtype=np.float32))
    W_in = np.ascontiguousarray(np.asarray(W_in, dtype=np.float32))
    b_in = np.ascontiguousarray(np.asarray(b_in, dtype=np.float32))
    W_out = np.ascontiguousarray(np.asarray(W_out, dtype=np.float32))
    b_out = np.ascontiguousarray(np.asarray(b_out, dtype=np.float32))

    nc = get_nc()
    in_maps = [
        {"X": X[i], "W_in": W_in, "b_in": b_in, "W_out": W_out, "b_out": b_out}
        for i in range(B)
    ]
    res = run_bass_kernel_spmd(nc, in_maps, core_ids=list(range(B)))
    return np.stack([res.results[i]["out"] for i in range(B)], axis=0)
